# revision 10
# baseline (speedup 1.0000x reference)
"""Trainium2 Bass kernel for nn_LocalProcessing (GNN message passing).

Self-contained: takes the full (unsharded) inputs of reference.setup_inputs(),
shards rows i over 8 NeuronCores, runs a Bass/Tile kernel via
bass_utils.run_bass_kernel_spmd, and returns the full output tuple.

Math: the all-pairs edge feature x[i,j] = [m_i/10, m_j, e_i, e_j] feeds two
Linear+ReLU layers.  x[i,j] @ W decomposes as A_i + B_j with per-node
projections A = [m/10, e, 1] @ [W_mi; W_ei; bias], B = [m, e] @ [W_mj; W_ej],
so the N^2-sized matmul collapses to small per-node matmuls plus elementwise
work over [i, j, k] handled on Scalar/Vector engines:
    aw = 1 / (relu(Aa_i + Ba_j)^2 + 0.01)
    ep = relu(Ae_i + Be_j)
    agg_i = e_i + sum_j(ep*aw) / sum_j(aw)
Each core owns 96 rows i, processed as 48 pair-blocks with SBUF layout
[partitions = (2 rows, 64 feats), free = 768 js].  One AllGather of the
updated embedding runs between the two message-passing iterations.
"""

import numpy as np

import concourse.bass as bass
import concourse.bacc as bacc
import concourse.mybir as mybir
import concourse.tile as tile
import concourse.bass_utils as bass_utils
from operator import add as _addop

from concourse import dve_ops
from concourse.dve_spec import Spec, Src0, Src1, C0, C1, C2, relu as _relu, sq as _sq, lower as _lower
from concourse.dve_spec import _has_src1 as _has_src1
from concourse.dve_uop import DveOpSpec


def _register_dve_op(name, spec):
    """Author a custom DVE op at runtime (tables are generated per-NEFF)."""
    for op in dve_ops.OPS:
        if op.name == name:
            return op
    row = max(dve_ops._SUB_OPCODE_FOR_NAME.values()) + 1
    assert row < 0x20
    dve_ops._SUB_OPCODE_FOR_NAME[name] = row
    shas = {}
    for ver in ("v3", "v4"):
        s = DveOpSpec(name=name, opcode=row, uops=_lower(spec, ver=ver),
                      rd1_en=_has_src1(spec))
        shas[ver] = s.sha(ver)
    op = dve_ops.DveOp(name, spec, subdim=False, uops_sha=shas)
    dve_ops.OPS.append(op)
    dve_ops.CUSTOM_DVE_SPECS[name] = spec
    return op


def _ref_sqsq(in0, in1, s0, s1, imm2):
    r = np.maximum(np.nan_to_num(in0.astype(np.float32) + s0), 0)
    return ((r * r + imm2) ** 2).astype(np.float32)


def _ref_relumr(in0, in1, s0, s1, imm2):
    r = np.maximum(np.nan_to_num(in0.astype(np.float32) + s0), 0)
    b = (r * in1).astype(np.float32)
    return b, s1 + b.reshape(b.shape[0], -1).sum(axis=-1, keepdims=True)


# t = (relu(in0 + s0)^2 + imm2)^2 — squared so ScalarE can apply 1/sqrt to
# recover aw = 1/(relu(u)^2 + imm2) in a single activation with den-accum.
SQSQ01 = _register_dve_op("SQSQ01_ANT", Spec(
    body=_sq(_sq(_relu(Src0 + C0)) + C2), reference=_ref_sqsq))
# out = relu(in0 + s0) * in1;  accum = s1 + sum(out)   (edge msg * weight)
RELUMR = _register_dve_op("RELUMR_ANT", Spec(
    body=_relu(Src0 + C0) * Src1, accum=_addop, accum_init=C1,
    reference=_ref_relumr))

N = 768
E = 64
MD = 6
NCORES = 8
PI = N // NCORES          # 96 rows per core
NPAIR = PI // 2           # 48 pair-blocks per core
F = N                     # free dim = all j

dt = mybir.dt
AF = mybir.ActivationFunctionType
ALU = mybir.AluOpType

TRACE = False
LAST_EXEC_NS = None
LAST_RESULTS = None

_compiled = {}


def _recip_act(nc, out, in_, bias_const):
    """out = 1/(in_ + bias).  Raw InstActivation: the bass wrapper bans
    Reciprocal generically; measured accuracy here is ~1e-5 relative on the
    positive, well-conditioned inputs this kernel produces."""
    eng = nc.scalar
    ins_ = [eng.lower_ap(in_)]
    for v in (bias_const, 1.0, 0.0):  # bias, scale, alpha
        ins_.append(mybir.ImmediateValue(dtype=dt.float32, value=float(v)))
    return eng.add_instruction(
        mybir.InstActivation(
            name=nc.get_next_instruction_name(),
            func=AF.Reciprocal,
            ins=ins_,
            outs=[eng.lower_ap(out)],
        )
    )


def _build(single=False):
    """single=True builds a 1-core variant with the collective replaced by a
    local DRAM bounce (same DMA shapes) — used for TimelineSim cost modeling."""
    nc = bacc.Bacc("TRN2", target_bir_lowering=False, debug=False,
                   num_devices=1 if single else NCORES)

    def di(name, shape):
        return nc.dram_tensor(name, list(shape), dt.float32,
                              kind="ExternalInput").ap()

    def do(name, shape):
        return nc.dram_tensor(name, list(shape), dt.float32,
                              kind="ExternalOutput").ap()

    mT_d = di("mT", (MD, N))
    ones_d = di("onesN", (1, N))
    eT0_d = di("eT0", (E, N))
    mTo_d = di("mT_own", (MD, PI))
    eTo_d = di("eT0_own", (E, PI))
    WBa_d = di("WBa", (71, 128))
    WBe_d = di("WBe", (71, 128))
    WAa1_d = di("WAa1", (71, 128))
    WAa2_d = di("WAa2", (71, 128))
    WCe1_d = di("WCe1", (71, 128))
    WCe2_d = di("WCe2", (71, 128))
    Wn1_d = di("Wn1", (72, E))
    bn1_d = di("bn1", (E, 1))
    Wn2b_d = di("Wn2b", (E + 1, E))
    Wp1_d = di("Wp1", (E, E))
    bp1_d = di("bp1", (E, 1))
    Wp2b_d = di("Wp2b", (E + 1, 2))
    Wt1_d = di("Wt1", (E, E))
    bt1_d = di("bt1", (E, 1))
    Wt2b_d = di("Wt2b", (E + 1, 2))
    Wo1_d = di("Wo1", (E, E))
    bo1_d = di("bo1", (E, 1))
    Wo2b_d = di("Wo2b", (E + 1, 2))
    Wf1_d = di("Wf1", (E + 1, E))
    bf1_d = di("bf1", (E, 1))
    Wf2b_d = di("Wf2b", (E + 1, 2))

    posT_d = do("posT", (2, PI))
    tanT_d = do("tanT", (2, PI))
    norT_d = do("norT", (2, PI))
    futT_d = do("futT", (2, PI))
    e2T_d = do("e2T", (E, PI))

    with tile.TileContext(nc) as tc:
        with (
            tc.tile_pool(name="const", bufs=1) as const,
            tc.tile_pool(name="big", bufs=2) as big,
            tc.tile_pool(name="small", bufs=2) as small,
            tc.tile_pool(name="sm2", bufs=3) as sm2,
            tc.tile_pool(name="pe2", bufs=2) as pe2,
            tc.tile_pool(name="pd", bufs=4) as pd,
            tc.tile_pool(name="pawb", bufs=4) as pawb,
            tc.tile_pool(name="ptr", bufs=2) as ptr,
            tc.tile_pool(name="psbt", bufs=2, space="PSUM") as psbt,
            tc.tile_pool(name="psdt", bufs=1, space="PSUM") as psdt,
            tc.tile_pool(name="pssm", bufs=2, space="PSUM") as pssm,
            tc.tile_pool(name="dram", bufs=1, space="DRAM") as dram,
        ):
            # ---- static setup -------------------------------------------
            rhs_all = const.tile([71, N], dt.float32)   # [m; e; 1] features, all nodes
            nc.sync.dma_start(rhs_all[0:MD, :], mT_d)
            nc.sync.dma_start(rhs_all[MD:MD + E, :], eT0_d)
            nc.sync.dma_start(rhs_all[70:71, :], ones_d)

            feat_own = const.tile([71, PI], dt.float32)  # own columns
            nc.sync.dma_start(feat_own[0:MD, :], mTo_d)
            nc.sync.dma_start(feat_own[MD:MD + E, :], eTo_d)
            nc.sync.dma_start(feat_own[70:71, :], ones_d[0:1, 0:PI])

            node_inT = const.tile([72, PI], dt.float32)
            nc.sync.dma_start(node_inT[0:MD, :], mTo_d)
            nc.sync.dma_start(node_inT[MD:MD + 2, :], mTo_d[0:2, :])

            eT_own1 = const.tile([E, PI], dt.float32)
            nc.sync.dma_start(eT_own1[:], eTo_d)

            _ntag = [0]

            def load_const(ap_d, shape):
                _ntag[0] += 1
                t = const.tile(list(shape), dt.float32, tag=f"w{_ntag[0]}")
                nc.sync.dma_start(t[:], ap_d)
                return t

            WBa = load_const(WBa_d, (71, 128))
            WBe = load_const(WBe_d, (71, 128))
            WAa1 = load_const(WAa1_d, (71, 128))
            WAa2 = load_const(WAa2_d, (71, 128))
            WCe1 = load_const(WCe1_d, (71, 128))
            WCe2 = load_const(WCe2_d, (71, 128))
            Wn1 = load_const(Wn1_d, (72, E))
            bn1 = load_const(bn1_d, (E, 1))
            Wn2b = load_const(Wn2b_d, (E + 1, E))
            heads = []
            for W1d, b1d, W2d in ((Wp1_d, bp1_d, Wp2b_d),
                                  (Wt1_d, bt1_d, Wt2b_d),
                                  (Wo1_d, bo1_d, Wo2b_d)):
                heads.append((load_const(W1d, (E, E)),
                              load_const(b1d, (E, 1)),
                              load_const(W2d, (E + 1, 2))))
            Wf1 = load_const(Wf1_d, (E + 1, E))
            bf1 = load_const(bf1_d, (E, 1))
            Wf2b = load_const(Wf2b_d, (E + 1, 2))

            e_prev = eT_own1  # [E, PI] tile holding current own embedding
            e2t = None

            for it in range(2):
                # ---- per-iteration projections --------------------------
                btps = psbt.tile([128, 1024], dt.float32)
                nc.tensor.matmul(btps[:, 0:512], WBa[:], rhs_all[:, 0:512])
                nc.tensor.matmul(btps[:, 512:768], WBa[:], rhs_all[:, 512:768])
                BT2 = big.tile([128, F], dt.float32)
                nc.scalar.copy(BT2[:], btps[:, 0:768])

                dtps = psdt.tile([128, 1024], dt.float32)
                nc.tensor.matmul(dtps[:, 0:512], WBe[:], rhs_all[:, 0:512])
                nc.tensor.matmul(dtps[:, 512:768], WBe[:], rhs_all[:, 512:768])
                DT2 = big.tile([128, F], dt.float32)
                nc.scalar.copy(DT2[:], dtps[:, 0:768])

                aps = pssm.tile([128, NPAIR], dt.float32, tag="sm")
                nc.tensor.matmul(aps[:], WAa1[:], feat_own[:, 0:NPAIR],
                                 start=True, stop=False)
                nc.tensor.matmul(aps[:], WAa2[:], feat_own[:, NPAIR:PI],
                                 start=False, stop=True)
                A_sb = small.tile([128, NPAIR], dt.float32)
                nc.scalar.copy(A_sb[:], aps[:])

                cps = pssm.tile([128, NPAIR], dt.float32, tag="sm")
                nc.tensor.matmul(cps[:], WCe1[:], feat_own[:, 0:NPAIR],
                                 start=True, stop=False)
                nc.tensor.matmul(cps[:], WCe2[:], feat_own[:, NPAIR:PI],
                                 start=False, stop=True)
                C_sb = small.tile([128, NPAIR], dt.float32)
                nc.scalar.copy(C_sb[:], cps[:])

                num_cols = small.tile([128, NPAIR], dt.float32)
                den_cols = small.tile([128, NPAIR], dt.float32)

                # ---- the N^2 elementwise sweep, 48 pair-blocks ----------
                # software-pipelined: V emits SQSQ for block b+1 before
                # RELUMR for block b, so V never waits on S's rsqrt.
                def emit_sqsq(b):
                    t = pd.tile([128, F], dt.float32, tag="t")
                    nc.vector._custom_dve(SQSQ01, out=t[:], in0=BT2[:],
                                          s0=A_sb[:, b:b + 1], s1=0.0,
                                          imm2=0.01)
                    return t

                def emit_rsqrt(b, t):
                    # aw = 1/sqrt(t) = 1/(relu(u)^2+0.01); den accum is free
                    aw_bf = pawb.tile([128, F], dt.bfloat16, tag="aw")
                    nc.scalar.activation(aw_bf[:], t[:],
                                         AF.Abs_reciprocal_sqrt,
                                         bias=0.0, scale=1.0,
                                         accum_out=den_cols[:, b:b + 1])
                    return aw_bf

                def emit_relumr(b, aw_bf):
                    tr = ptr.tile([128, F], dt.bfloat16, tag="tr")
                    nc.vector._custom_dve(RELUMR, out=tr[:], in0=DT2[:],
                                          in1=aw_bf[:],
                                          s0=C_sb[:, b:b + 1], s1=0.0,
                                          accum_out=num_cols[:, b:b + 1])

                t_prev = emit_sqsq(0)
                aw_prev = emit_rsqrt(0, t_prev)
                for b in range(1, NPAIR):
                    t_cur = emit_sqsq(b)
                    emit_relumr(b - 1, aw_prev)
                    aw_prev = emit_rsqrt(b, t_cur)
                emit_relumr(NPAIR - 1, aw_prev)

                # ---- aggregate + node MLP -------------------------------
                numT = sm2.tile([E, PI], dt.float32)
                nc.sync.dma_start(numT[:, 0:NPAIR], num_cols[0:E, :])
                nc.sync.dma_start(numT[:, NPAIR:PI], num_cols[E:128, :])
                denT = sm2.tile([E, PI], dt.float32)
                nc.sync.dma_start(denT[:, 0:NPAIR], den_cols[0:E, :])
                nc.sync.dma_start(denT[:, NPAIR:PI], den_cols[E:128, :])
                recT = sm2.tile([E, PI], dt.float32)
                nc.vector.reciprocal_approx_fast(recT[:], denT[:])
                prodT = sm2.tile([E, PI], dt.float32)
                nc.vector.tensor_tensor(prodT[:], numT[:], recT[:], op=ALU.mult)
                aggT = sm2.tile([E, PI], dt.float32)
                nc.vector.tensor_tensor(aggT[:], prodT[:], e_prev[0:E, :],
                                        op=ALU.add)
                nc.sync.dma_start(node_inT[8:72, :], aggT[:])

                nhps = pssm.tile([E, PI], dt.float32, tag="sm")
                nc.tensor.matmul(nhps[:], Wn1[:], node_inT[0:72, :])
                nh = sm2.tile([E + 1, PI], dt.float32)
                nc.scalar.activation(nh[0:E, :], nhps[:], AF.Relu,
                                     bias=bn1[:], scale=1.0)
                nc.vector.memset(nh[E:E + 1, :], 1.0)
                e2ps = pssm.tile([E, PI], dt.float32, tag="sm")
                nc.tensor.matmul(e2ps[:], Wn2b[:], nh[0:E + 1, :])
                e2t = pe2.tile([E + 1, PI], dt.float32)
                nc.scalar.copy(e2t[0:E, :], e2ps[:])
                nc.vector.memset(e2t[E:E + 1, :], 1.0)

                if it == 0:
                    # all-gather updated embeddings for iteration 2
                    ag_in = dram.tile([E, PI], dt.float32)
                    if single:
                        ag_out = dram.tile([E * NCORES, PI], dt.float32)
                    else:
                        ag_out = dram.tile([E * NCORES, PI], dt.float32,
                                           addr_space="Shared")
                    nc.sync.dma_start(ag_in[:], e2t[0:E, :])
                    if single:
                        nc.sync.dma_start(ag_out[0:E, :], ag_in[:])
                    else:
                        nc.gpsimd.collective_compute(
                            "AllGather", ALU.bypass,
                            replica_groups=[list(range(NCORES))],
                            ins=[ag_in.opt()], outs=[ag_out.opt()],
                        )
                    for g in range(NCORES):
                        nc.sync.dma_start(rhs_all[MD:MD + E, PI * g:PI * (g + 1)],
                                          ag_out[E * g:E * (g + 1), :])
                    nc.sync.dma_start(feat_own[MD:MD + E, :], e2t[0:E, :])
                    e_prev = e2t

            # ---- heads ----------------------------------------------------
            for (W1, b1, W2b), out_d in zip(heads, (posT_d, tanT_d, norT_d)):
                hps = pssm.tile([E, PI], dt.float32, tag="sm")
                nc.tensor.matmul(hps[:], W1[:], e2t[0:E, :])
                h = sm2.tile([E + 1, PI], dt.float32)
                nc.scalar.activation(h[0:E, :], hps[:], AF.Relu,
                                     bias=b1[:], scale=1.0)
                nc.vector.memset(h[E:E + 1, :], 1.0)
                ops2 = pssm.tile([2, PI], dt.float32, tag="sm")
                nc.tensor.matmul(ops2[:], W2b[:], h[0:E + 1, :])
                osb = sm2.tile([2, PI], dt.float32)
                nc.scalar.copy(osb[:], ops2[:])
                nc.sync.dma_start(out_d, osb[:])

            # future-position head reads [e2, 1] directly (row E is ones)
            fps = pssm.tile([E, PI], dt.float32, tag="sm")
            nc.tensor.matmul(fps[:], Wf1[:], e2t[0:E + 1, :])
            fh = sm2.tile([E + 1, PI], dt.float32)
            nc.scalar.activation(fh[0:E, :], fps[:], AF.Relu,
                                 bias=bf1[:], scale=1.0)
            nc.vector.memset(fh[E:E + 1, :], 1.0)
            fps2 = pssm.tile([2, PI], dt.float32, tag="sm")
            nc.tensor.matmul(fps2[:], Wf2b[:], fh[0:E + 1, :])
            fsb = sm2.tile([2, PI], dt.float32)
            nc.scalar.copy(fsb[:], fps2[:])
            nc.sync.dma_start(futT_d, fsb[:])

            e2sb = sm2.tile([E, PI], dt.float32)
            nc.vector.tensor_copy(e2sb[:], e2t[0:E, :])
            nc.sync.dma_start(e2T_d, e2sb[:])

    nc.compile()
    return nc


def kernel(neighborhood, neighborhood_embedding, representative_id,
           Wa, ba, We, be, Wn1, bn1, Wn2, bn2,
           Wp1, bp1, Wp2, bp2, Wt1, bt1, Wt2, bt2, Wo1, bo1, Wo2, bo2,
           Wf1, bf1, Wf2, bf2):
    global LAST_EXEC_NS, LAST_RESULTS

    f32 = np.float32
    m = np.asarray(neighborhood, f32)
    e0 = np.asarray(neighborhood_embedding, f32)
    rep = int(np.asarray(representative_id))
    Wa = np.asarray(Wa, f32); ba = np.asarray(ba, f32)
    We = np.asarray(We, f32); be = np.asarray(be, f32)
    Wn1 = np.asarray(Wn1, f32); bn1 = np.asarray(bn1, f32)
    Wn2 = np.asarray(Wn2, f32); bn2 = np.asarray(bn2, f32)

    def blockify(W, b):
        # per-node projection weights for [m; e; 1] features
        WA = np.vstack([W[0:MD] / 10.0, W[MD + MD:MD + MD + E], b[None, :]])
        WB = np.vstack([W[MD:2 * MD], W[2 * MD + E:], np.zeros((1, E), f32)])
        return WA.astype(f32), WB.astype(f32)

    WAblk, WBblk = blockify(Wa, ba)
    WCblk, WDblk = blockify(We, be)
    z = np.zeros_like(WAblk)
    shared = {
        "mT": np.ascontiguousarray(m.T),
        "onesN": np.ones((1, N), f32),
        "eT0": np.ascontiguousarray(e0.T),
        "WBa": np.ascontiguousarray(np.concatenate([WBblk, WBblk], axis=1)),
        "WBe": np.ascontiguousarray(np.concatenate([WDblk, WDblk], axis=1)),
        "WAa1": np.ascontiguousarray(np.concatenate([WAblk, z], axis=1)),
        "WAa2": np.ascontiguousarray(np.concatenate([z, WAblk], axis=1)),
        "WCe1": np.ascontiguousarray(np.concatenate([WCblk, z], axis=1)),
        "WCe2": np.ascontiguousarray(np.concatenate([z, WCblk], axis=1)),
        "Wn1": np.ascontiguousarray(Wn1),
        "bn1": np.ascontiguousarray(
            (bn1 - m[rep, :2] @ Wn1[MD:MD + 2]).reshape(E, 1)),
        "Wn2b": np.vstack([Wn2, np.asarray(bn2, f32)[None, :]]),
        "Wp1": np.asarray(Wp1, f32),
        "bp1": np.asarray(bp1, f32).reshape(E, 1),
        "Wp2b": np.vstack([np.asarray(Wp2, f32), np.asarray(bp2, f32)[None, :]]),
        "Wt1": np.asarray(Wt1, f32),
        "bt1": np.asarray(bt1, f32).reshape(E, 1),
        "Wt2b": np.vstack([np.asarray(Wt2, f32), np.asarray(bt2, f32)[None, :]]),
        "Wo1": np.asarray(Wo1, f32),
        "bo1": np.asarray(bo1, f32).reshape(E, 1),
        "Wo2b": np.vstack([np.asarray(Wo2, f32), np.asarray(bo2, f32)[None, :]]),
        "Wf1": np.asarray(Wf1, f32),
        "bf1": np.asarray(bf1, f32).reshape(E, 1),
        "Wf2b": np.vstack([np.asarray(Wf2, f32), np.asarray(bf2, f32)[None, :]]),
    }
    shared = {k: np.ascontiguousarray(v.astype(f32)) for k, v in shared.items()}

    in_maps = []
    for c in range(NCORES):
        sl = slice(PI * c, PI * (c + 1))
        im = dict(shared)
        im["mT_own"] = np.ascontiguousarray(m.T[:, sl])
        im["eT0_own"] = np.ascontiguousarray(e0.T[:, sl])
        in_maps.append(im)

    if "nc" not in _compiled:
        _compiled["nc"] = _build()
    nc = _compiled["nc"]

    kw = {}
    if TRACE:
        kw = dict(trace=True, trace_cores=list(range(NCORES)),
                  stitch_traces=True)
    res = bass_utils.run_bass_kernel_spmd(
        nc, in_maps, core_ids=list(range(NCORES)), **kw)
    LAST_EXEC_NS = res.exec_time_ns
    LAST_RESULTS = res

    pos = np.concatenate([res.results[c]["posT"].T for c in range(NCORES)])
    tan = np.concatenate([res.results[c]["tanT"].T for c in range(NCORES)])
    nor = np.concatenate([res.results[c]["norT"].T for c in range(NCORES)])
    fut = np.concatenate([res.results[c]["futT"].T for c in range(NCORES)])
    e2 = np.concatenate([res.results[c]["e2T"].T for c in range(NCORES)])

    return (m[rep:rep + 1, :].copy(), e2[rep:rep + 1, :].copy(),
            pos, tan, nor, fut, fut.copy())


def timeline_estimate_ns(trace=False):
    """Cost-model estimate of per-core HW exec time (single-core build,
    collective replaced by a local DRAM bounce)."""
    from concourse.timeline_sim import TimelineSim
    nc = _build(single=True)
    tl = TimelineSim(nc, trace=trace)
    total = tl.simulate()
    return total, tl


# revision 13
# speedup vs baseline: 1.0005x; 1.0005x over previous
"""Trainium2 Bass kernel for nn_LocalProcessing (GNN message passing).

Self-contained: takes the full (unsharded) inputs of reference.setup_inputs(),
shards rows i over 8 NeuronCores, runs a Bass/Tile kernel via
bass_utils.run_bass_kernel_spmd, and returns the full output tuple.

Math: the all-pairs edge feature x[i,j] = [m_i/10, m_j, e_i, e_j] feeds two
Linear+ReLU layers.  x[i,j] @ W decomposes as A_i + B_j with per-node
projections A = [m/10, e, 1] @ [W_mi; W_ei; bias], B = [m, e] @ [W_mj; W_ej],
so the N^2-sized matmul collapses to small per-node matmuls plus elementwise
work over [i, j, k] handled on Scalar/Vector engines:
    aw = 1 / (relu(Aa_i + Ba_j)^2 + 0.01)
    ep = relu(Ae_i + Be_j)
    agg_i = e_i + sum_j(ep*aw) / sum_j(aw)
Each core owns 96 rows i, processed as 48 pair-blocks with SBUF layout
[partitions = (2 rows, 64 feats), free = 768 js].  One AllGather of the
updated embedding runs between the two message-passing iterations.
"""

import numpy as np

import concourse.bass as bass
import concourse.bacc as bacc
import concourse.mybir as mybir
import concourse.tile as tile
import concourse.bass_utils as bass_utils
from operator import add as _addop

from concourse import dve_ops
from concourse.dve_spec import Spec, Src0, Src1, C0, C1, C2, relu as _relu, sq as _sq, lower as _lower
from concourse.dve_spec import _has_src1 as _has_src1
from concourse.dve_uop import DveOpSpec


def _register_dve_op(name, spec):
    """Author a custom DVE op at runtime (tables are generated per-NEFF)."""
    for op in dve_ops.OPS:
        if op.name == name:
            return op
    row = max(dve_ops._SUB_OPCODE_FOR_NAME.values()) + 1
    assert row < 0x20
    dve_ops._SUB_OPCODE_FOR_NAME[name] = row
    shas = {}
    for ver in ("v3", "v4"):
        s = DveOpSpec(name=name, opcode=row, uops=_lower(spec, ver=ver),
                      rd1_en=_has_src1(spec))
        shas[ver] = s.sha(ver)
    op = dve_ops.DveOp(name, spec, subdim=False, uops_sha=shas)
    dve_ops.OPS.append(op)
    dve_ops.CUSTOM_DVE_SPECS[name] = spec
    return op


def _ref_sqsq(in0, in1, s0, s1, imm2):
    r = np.maximum(np.nan_to_num(in0.astype(np.float32) + s0), 0)
    return ((r * r + imm2) ** 2).astype(np.float32)


def _ref_relumr(in0, in1, s0, s1, imm2):
    r = np.maximum(np.nan_to_num(in0.astype(np.float32) + s0), 0)
    b = (r * in1).astype(np.float32)
    return b, s1 + b.reshape(b.shape[0], -1).sum(axis=-1, keepdims=True)


# t = (relu(in0 + s0)^2 + imm2)^2 — squared so ScalarE can apply 1/sqrt to
# recover aw = 1/(relu(u)^2 + imm2) in a single activation with den-accum.
SQSQ01 = _register_dve_op("SQSQ01_ANT", Spec(
    body=_sq(_sq(_relu(Src0 + C0)) + C2), reference=_ref_sqsq))
# out = relu(in0 + s0) * in1;  accum = s1 + sum(out)   (edge msg * weight)
RELUMR = _register_dve_op("RELUMR_ANT", Spec(
    body=_relu(Src0 + C0) * Src1, accum=_addop, accum_init=C1,
    reference=_ref_relumr))

N = 768
E = 64
MD = 6
NCORES = 8
PI = N // NCORES          # 96 rows per core
NPAIR = PI // 2           # 48 pair-blocks per core
F = N                     # free dim = all j

dt = mybir.dt
AF = mybir.ActivationFunctionType
ALU = mybir.AluOpType

TRACE = False
LAST_EXEC_NS = None
LAST_RESULTS = None

_compiled = {}


def _recip_act(nc, out, in_, bias_const):
    """out = 1/(in_ + bias).  Raw InstActivation: the bass wrapper bans
    Reciprocal generically; measured accuracy here is ~1e-5 relative on the
    positive, well-conditioned inputs this kernel produces."""
    eng = nc.scalar
    ins_ = [eng.lower_ap(in_)]
    for v in (bias_const, 1.0, 0.0):  # bias, scale, alpha
        ins_.append(mybir.ImmediateValue(dtype=dt.float32, value=float(v)))
    return eng.add_instruction(
        mybir.InstActivation(
            name=nc.get_next_instruction_name(),
            func=AF.Reciprocal,
            ins=ins_,
            outs=[eng.lower_ap(out)],
        )
    )


def _build(single=False):
    """single=True builds a 1-core variant with the collective replaced by a
    local DRAM bounce (same DMA shapes) — used for TimelineSim cost modeling."""
    nc = bacc.Bacc("TRN2", target_bir_lowering=False, debug=False,
                   num_devices=1 if single else NCORES)

    def di(name, shape):
        return nc.dram_tensor(name, list(shape), dt.float32,
                              kind="ExternalInput").ap()

    def do(name, shape):
        return nc.dram_tensor(name, list(shape), dt.float32,
                              kind="ExternalOutput").ap()

    mT_d = di("mT", (MD, N))
    ones_d = di("onesN", (1, N))
    eT0_d = di("eT0", (E, N))
    mTo_d = di("mT_own", (MD, PI))
    eTo_d = di("eT0_own", (E, PI))
    W71_d = di("W71", (71, 768))    # [WBa|WBe|WAa1|WAa2|WCe1|WCe2]
    Wn1_d = di("Wn1", (72, E))      # node-MLP L1, rows reordered [agg|m|m2]
    W65_d = di("W65", (65, 136))    # [Wn2b|Wp2b|Wt2b|Wo2b|Wf1|Wf2b]
    W64_d = di("W64", (64, 197))    # [Wp1|Wt1|Wo1|bn1|bp1|bt1|bo1|bf1]

    posT_d = do("posT", (2, PI))
    tanT_d = do("tanT", (2, PI))
    norT_d = do("norT", (2, PI))
    futT_d = do("futT", (2, PI))
    e2T_d = do("e2T", (E, PI))

    with tile.TileContext(nc) as tc:
        with (
            tc.tile_pool(name="const", bufs=1) as const,
            tc.tile_pool(name="big", bufs=2) as big,
            tc.tile_pool(name="small", bufs=2) as small,
            tc.tile_pool(name="sm2", bufs=3) as sm2,
            tc.tile_pool(name="pe2", bufs=2) as pe2,
            tc.tile_pool(name="pd", bufs=4) as pd,
            tc.tile_pool(name="pawb", bufs=4) as pawb,
            tc.tile_pool(name="ptr", bufs=2) as ptr,
            tc.tile_pool(name="psbt", bufs=2, space="PSUM") as psbt,
            tc.tile_pool(name="psdt", bufs=1, space="PSUM") as psdt,
            tc.tile_pool(name="pssm", bufs=2, space="PSUM") as pssm,
            tc.tile_pool(name="dram", bufs=1, space="DRAM") as dram,
        ):
            # ---- static setup -------------------------------------------
            rhs_all = const.tile([71, N], dt.float32)   # [m; e; 1] features, all nodes
            nc.sync.dma_start(rhs_all[0:MD, :], mT_d)
            nc.sync.dma_start(rhs_all[MD:MD + E, :], eT0_d)
            nc.sync.dma_start(rhs_all[70:71, :], ones_d)

            feat_own = const.tile([71, PI], dt.float32)  # own columns
            nc.sync.dma_start(feat_own[0:MD, :], mTo_d)
            nc.sync.dma_start(feat_own[MD:MD + E, :], eTo_d)
            nc.sync.dma_start(feat_own[70:71, :], ones_d[0:1, 0:PI])

            node_inT = const.tile([72, PI], dt.float32)
            nc.sync.dma_start(node_inT[E:E + MD, :], mTo_d)
            nc.sync.dma_start(node_inT[E + MD:72, :], mTo_d[0:2, :])

            eT_own1 = const.tile([E, PI], dt.float32)
            nc.sync.dma_start(eT_own1[:], eTo_d)

            W71 = const.tile([71, 768], dt.float32)
            nc.sync.dma_start(W71[:], W71_d)
            Wn1 = const.tile([72, E], dt.float32)
            nc.sync.dma_start(Wn1[:], Wn1_d)
            W65 = const.tile([65, 136], dt.float32)
            nc.sync.dma_start(W65[:], W65_d)
            W64 = const.tile([64, 197], dt.float32)
            nc.sync.dma_start(W64[:], W64_d)
            WBa, WBe = W71[:, 0:128], W71[:, 128:256]
            WAa1, WAa2 = W71[:, 256:384], W71[:, 384:512]
            WCe1, WCe2 = W71[:, 512:640], W71[:, 640:768]
            Wn2b, Wf1 = W65[:, 0:64], W65[:, 70:134]
            bn1 = W64[:, 192:193]
            heads = [(W64[:, 0:64], W64[:, 193:194], W65[:, 64:66]),
                     (W64[:, 64:128], W64[:, 194:195], W65[:, 66:68]),
                     (W64[:, 128:192], W64[:, 195:196], W65[:, 68:70])]
            bf1 = W64[:, 196:197]
            Wf2b = W65[:, 134:136]

            e_prev = eT_own1  # [E, PI] tile holding current own embedding
            e2t = None

            for it in range(2):
                # ---- per-iteration projections --------------------------
                btps = psbt.tile([128, 1024], dt.float32)
                nc.tensor.matmul(btps[:, 0:512], WBa, rhs_all[:, 0:512])
                nc.tensor.matmul(btps[:, 512:768], WBa, rhs_all[:, 512:768])
                BT2 = big.tile([128, F], dt.float32)
                nc.scalar.copy(BT2[:], btps[:, 0:768])

                dtps = psdt.tile([128, 1024], dt.float32)
                nc.tensor.matmul(dtps[:, 0:512], WBe, rhs_all[:, 0:512])
                nc.tensor.matmul(dtps[:, 512:768], WBe, rhs_all[:, 512:768])
                DT2 = big.tile([128, F], dt.float32)
                nc.scalar.copy(DT2[:], dtps[:, 0:768])

                aps = pssm.tile([128, NPAIR], dt.float32, tag="sm")
                nc.tensor.matmul(aps[:], WAa1, feat_own[:, 0:NPAIR],
                                 start=True, stop=False)
                nc.tensor.matmul(aps[:], WAa2, feat_own[:, NPAIR:PI],
                                 start=False, stop=True)
                A_sb = small.tile([128, NPAIR], dt.float32)
                nc.scalar.copy(A_sb[:], aps[:])

                cps = pssm.tile([128, NPAIR], dt.float32, tag="sm")
                nc.tensor.matmul(cps[:], WCe1, feat_own[:, 0:NPAIR],
                                 start=True, stop=False)
                nc.tensor.matmul(cps[:], WCe2, feat_own[:, NPAIR:PI],
                                 start=False, stop=True)
                C_sb = small.tile([128, NPAIR], dt.float32)
                nc.scalar.copy(C_sb[:], cps[:])

                num_cols = small.tile([128, NPAIR], dt.float32)
                den_cols = small.tile([128, NPAIR], dt.float32)

                # ---- the N^2 elementwise sweep, 48 pair-blocks ----------
                # software-pipelined: V emits SQSQ for block b+1 before
                # RELUMR for block b, so V never waits on S's rsqrt.
                def emit_sqsq(b):
                    t = pd.tile([128, F], dt.float32, tag="t")
                    nc.vector._custom_dve(SQSQ01, out=t[:], in0=BT2[:],
                                          s0=A_sb[:, b:b + 1], s1=0.0,
                                          imm2=0.01)
                    return t

                def emit_rsqrt(b, t):
                    # aw = 1/sqrt(t) = 1/(relu(u)^2+0.01); den accum is free
                    aw_bf = pawb.tile([128, F], dt.bfloat16, tag="aw")
                    nc.scalar.activation(aw_bf[:], t[:],
                                         AF.Abs_reciprocal_sqrt,
                                         bias=0.0, scale=1.0,
                                         accum_out=den_cols[:, b:b + 1])
                    return aw_bf

                def emit_relumr(b, aw_bf):
                    tr = ptr.tile([128, F], dt.bfloat16, tag="tr")
                    nc.vector._custom_dve(RELUMR, out=tr[:], in0=DT2[:],
                                          in1=aw_bf[:],
                                          s0=C_sb[:, b:b + 1], s1=0.0,
                                          accum_out=num_cols[:, b:b + 1])

                t_prev = emit_sqsq(0)
                aw_prev = emit_rsqrt(0, t_prev)
                for b in range(1, NPAIR):
                    t_cur = emit_sqsq(b)
                    emit_relumr(b - 1, aw_prev)
                    aw_prev = emit_rsqrt(b, t_cur)
                emit_relumr(NPAIR - 1, aw_prev)

                # ---- aggregate + node MLP -------------------------------
                rec_cols = sm2.tile([128, NPAIR], dt.float32)
                nc.vector.reciprocal_approx_fast(rec_cols[:], den_cols[:])
                q_cols = sm2.tile([128, NPAIR], dt.float32)
                nc.vector.tensor_tensor(q_cols[:], num_cols[:], rec_cols[:],
                                        op=ALU.mult)
                qT = sm2.tile([E, PI], dt.float32)
                nc.sync.dma_start(qT[:, 0:NPAIR], q_cols[0:E, :])
                nc.sync.dma_start(qT[:, NPAIR:PI], q_cols[E:128, :])
                # agg -> node_inT rows 0:64 (Wn1 rows reordered to match)
                nc.vector.tensor_tensor(node_inT[0:E, :], qT[:],
                                        e_prev[0:E, :], op=ALU.add)

                nhps = pssm.tile([E, PI], dt.float32, tag="sm")
                nc.tensor.matmul(nhps[:], Wn1[:], node_inT[0:72, :])
                nh = sm2.tile([E + 1, PI], dt.float32)
                nc.scalar.activation(nh[0:E, :], nhps[:], AF.Relu,
                                     bias=bn1, scale=1.0)
                nc.vector.memset(nh[E:E + 1, :], 1.0)
                e2ps = pssm.tile([E, PI], dt.float32, tag="sm")
                nc.tensor.matmul(e2ps[:], Wn2b, nh[0:E + 1, :])
                e2t = pe2.tile([E + 1, PI], dt.float32)
                nc.scalar.copy(e2t[0:E, :], e2ps[:])
                nc.vector.memset(e2t[E:E + 1, :], 1.0)

                if it == 0:
                    # all-gather updated embeddings for iteration 2
                    ag_in = dram.tile([E, PI], dt.float32)
                    if single:
                        ag_out = dram.tile([E * NCORES, PI], dt.float32)
                    else:
                        ag_out = dram.tile([E * NCORES, PI], dt.float32,
                                           addr_space="Shared")
                    nc.sync.dma_start(ag_in[:], e2t[0:E, :])
                    if single:
                        nc.sync.dma_start(ag_out[0:E, :], ag_in[:])
                    else:
                        nc.gpsimd.collective_compute(
                            "AllGather", ALU.bypass,
                            replica_groups=[list(range(NCORES))],
                            ins=[ag_in.opt()], outs=[ag_out.opt()],
                        )
                    nc.sync.dma_start(
                        rhs_all[MD:MD + E, :].rearrange("k (g c) -> k g c",
                                                        g=NCORES),
                        ag_out[:].rearrange("(g k) c -> k g c", k=E))
                    nc.sync.dma_start(feat_own[MD:MD + E, :], e2t[0:E, :])
                    e_prev = e2t

            # ---- heads ----------------------------------------------------
            for (W1, b1, W2b), out_d in zip(heads, (posT_d, tanT_d, norT_d)):
                hps = pssm.tile([E, PI], dt.float32, tag="sm")
                nc.tensor.matmul(hps[:], W1, e2t[0:E, :])
                h = sm2.tile([E + 1, PI], dt.float32)
                nc.scalar.activation(h[0:E, :], hps[:], AF.Relu,
                                     bias=b1, scale=1.0)
                nc.vector.memset(h[E:E + 1, :], 1.0)
                ops2 = pssm.tile([2, PI], dt.float32, tag="sm")
                nc.tensor.matmul(ops2[:], W2b, h[0:E + 1, :])
                osb = sm2.tile([2, PI], dt.float32)
                nc.scalar.copy(osb[:], ops2[:])
                nc.sync.dma_start(out_d, osb[:])

            # future-position head reads [e2, 1] directly (row E is ones)
            fps = pssm.tile([E, PI], dt.float32, tag="sm")
            nc.tensor.matmul(fps[:], Wf1, e2t[0:E + 1, :])
            fh = sm2.tile([E + 1, PI], dt.float32)
            nc.scalar.activation(fh[0:E, :], fps[:], AF.Relu,
                                 bias=bf1, scale=1.0)
            nc.vector.memset(fh[E:E + 1, :], 1.0)
            fps2 = pssm.tile([2, PI], dt.float32, tag="sm")
            nc.tensor.matmul(fps2[:], Wf2b, fh[0:E + 1, :])
            fsb = sm2.tile([2, PI], dt.float32)
            nc.scalar.copy(fsb[:], fps2[:])
            nc.sync.dma_start(futT_d, fsb[:])

            nc.sync.dma_start(e2T_d, e2t[0:E, :])

    nc.compile()
    return nc


def kernel(neighborhood, neighborhood_embedding, representative_id,
           Wa, ba, We, be, Wn1, bn1, Wn2, bn2,
           Wp1, bp1, Wp2, bp2, Wt1, bt1, Wt2, bt2, Wo1, bo1, Wo2, bo2,
           Wf1, bf1, Wf2, bf2):
    global LAST_EXEC_NS, LAST_RESULTS

    f32 = np.float32
    m = np.asarray(neighborhood, f32)
    e0 = np.asarray(neighborhood_embedding, f32)
    rep = int(np.asarray(representative_id))
    Wa = np.asarray(Wa, f32); ba = np.asarray(ba, f32)
    We = np.asarray(We, f32); be = np.asarray(be, f32)
    Wn1 = np.asarray(Wn1, f32); bn1 = np.asarray(bn1, f32)
    Wn2 = np.asarray(Wn2, f32); bn2 = np.asarray(bn2, f32)

    def blockify(W, b):
        # per-node projection weights for [m; e; 1] features
        WA = np.vstack([W[0:MD] / 10.0, W[MD + MD:MD + MD + E], b[None, :]])
        WB = np.vstack([W[MD:2 * MD], W[2 * MD + E:], np.zeros((1, E), f32)])
        return WA.astype(f32), WB.astype(f32)

    WAblk, WBblk = blockify(Wa, ba)
    WCblk, WDblk = blockify(We, be)
    z = np.zeros_like(WAblk)
    # columns: [WBa(128) | WBe(128) | WAa1(128) | WAa2(128) | WCe1(128) | WCe2(128)]
    W71 = np.concatenate([
        np.concatenate([WBblk, WBblk], axis=1),
        np.concatenate([WDblk, WDblk], axis=1),
        np.concatenate([WAblk, z], axis=1),
        np.concatenate([z, WAblk], axis=1),
        np.concatenate([WCblk, z], axis=1),
        np.concatenate([z, WCblk], axis=1)], axis=1)
    bn1_adj = (bn1 - m[rep, :2] @ Wn1[MD:MD + 2]).reshape(E, 1)
    Wn1_r = np.vstack([Wn1[MD + 2:], Wn1[0:MD], Wn1[MD:MD + 2]])  # [agg|m|m2]
    W65 = np.concatenate([
        np.vstack([Wn2, np.asarray(bn2, f32)[None, :]]),
        np.vstack([np.asarray(Wp2, f32), np.asarray(bp2, f32)[None, :]]),
        np.vstack([np.asarray(Wt2, f32), np.asarray(bt2, f32)[None, :]]),
        np.vstack([np.asarray(Wo2, f32), np.asarray(bo2, f32)[None, :]]),
        np.asarray(Wf1, f32),
        np.vstack([np.asarray(Wf2, f32), np.asarray(bf2, f32)[None, :]])], axis=1)
    W64 = np.concatenate([
        np.asarray(Wp1, f32), np.asarray(Wt1, f32), np.asarray(Wo1, f32),
        bn1_adj,
        np.asarray(bp1, f32).reshape(E, 1),
        np.asarray(bt1, f32).reshape(E, 1),
        np.asarray(bo1, f32).reshape(E, 1),
        np.asarray(bf1, f32).reshape(E, 1)], axis=1)
    shared = {
        "mT": m.T, "onesN": np.ones((1, N), f32), "eT0": e0.T,
        "W71": W71, "Wn1": Wn1_r, "W65": W65, "W64": W64,
    }
    shared = {k: np.ascontiguousarray(v.astype(f32)) for k, v in shared.items()}

    in_maps = []
    for c in range(NCORES):
        sl = slice(PI * c, PI * (c + 1))
        im = dict(shared)
        im["mT_own"] = np.ascontiguousarray(m.T[:, sl])
        im["eT0_own"] = np.ascontiguousarray(e0.T[:, sl])
        in_maps.append(im)

    if "nc" not in _compiled:
        _compiled["nc"] = _build()
    nc = _compiled["nc"]

    kw = {}
    if TRACE:
        kw = dict(trace=True, trace_cores=list(range(NCORES)),
                  stitch_traces=True)
    res = bass_utils.run_bass_kernel_spmd(
        nc, in_maps, core_ids=list(range(NCORES)), **kw)
    LAST_EXEC_NS = res.exec_time_ns
    LAST_RESULTS = res

    pos = np.concatenate([res.results[c]["posT"].T for c in range(NCORES)])
    tan = np.concatenate([res.results[c]["tanT"].T for c in range(NCORES)])
    nor = np.concatenate([res.results[c]["norT"].T for c in range(NCORES)])
    fut = np.concatenate([res.results[c]["futT"].T for c in range(NCORES)])
    e2 = np.concatenate([res.results[c]["e2T"].T for c in range(NCORES)])

    return (m[rep:rep + 1, :].copy(), e2[rep:rep + 1, :].copy(),
            pos, tan, nor, fut, fut.copy())


def timeline_estimate_ns(trace=False):
    """Cost-model estimate of per-core HW exec time (single-core build,
    collective replaced by a local DRAM bounce)."""
    from concourse.timeline_sim import TimelineSim
    nc = _build(single=True)
    tl = TimelineSim(nc, trace=trace)
    total = tl.simulate()
    return total, tl


# revision 15
# speedup vs baseline: 1.0662x; 1.0657x over previous
"""Trainium2 Bass kernel for nn_LocalProcessing (GNN message passing).

Self-contained: takes the full (unsharded) inputs of reference.setup_inputs(),
shards rows i over 8 NeuronCores, runs a Bass/Tile kernel via
bass_utils.run_bass_kernel_spmd, and returns the full output tuple.

Math: the all-pairs edge feature x[i,j] = [m_i/10, m_j, e_i, e_j] feeds two
Linear+ReLU layers.  x[i,j] @ W decomposes as A_i + B_j with per-node
projections A = [m/10, e, 1] @ [W_mi; W_ei; bias], B = [m, e] @ [W_mj; W_ej],
so the N^2-sized matmul collapses to small per-node matmuls plus elementwise
work over [i, j, k] handled on Scalar/Vector engines:
    aw = 1 / (relu(Aa_i + Ba_j)^2 + 0.01)
    ep = relu(Ae_i + Be_j)
    agg_i = e_i + sum_j(ep*aw) / sum_j(aw)
Each core owns 96 rows i, processed as 48 pair-blocks with SBUF layout
[partitions = (2 rows, 64 feats), free = 768 js].  One AllGather of the
updated embedding runs between the two message-passing iterations.
"""

import numpy as np

import concourse.bass as bass
import concourse.bacc as bacc
import concourse.mybir as mybir
import concourse.tile as tile
import concourse.bass_utils as bass_utils
from operator import add as _addop

from concourse import dve_ops
from concourse.dve_spec import Spec, Src0, Src1, C0, C1, C2, relu as _relu, sq as _sq, lower as _lower
from concourse.dve_spec import _has_src1 as _has_src1
from concourse.dve_uop import DveOpSpec


def _register_dve_op(name, spec):
    """Author a custom DVE op at runtime (tables are generated per-NEFF)."""
    for op in dve_ops.OPS:
        if op.name == name:
            return op
    row = max(dve_ops._SUB_OPCODE_FOR_NAME.values()) + 1
    assert row < 0x20
    dve_ops._SUB_OPCODE_FOR_NAME[name] = row
    shas = {}
    for ver in ("v3", "v4"):
        s = DveOpSpec(name=name, opcode=row, uops=_lower(spec, ver=ver),
                      rd1_en=_has_src1(spec))
        shas[ver] = s.sha(ver)
    op = dve_ops.DveOp(name, spec, subdim=False, uops_sha=shas)
    dve_ops.OPS.append(op)
    dve_ops.CUSTOM_DVE_SPECS[name] = spec
    return op


def _ref_sqsq(in0, in1, s0, s1, imm2):
    r = np.maximum(np.nan_to_num(in0.astype(np.float32) + s0), 0)
    return ((r * r + imm2) ** 2).astype(np.float32)


def _ref_relumr(in0, in1, s0, s1, imm2):
    r = np.maximum(np.nan_to_num(in0.astype(np.float32) + s0), 0)
    b = (r * in1).astype(np.float32)
    return b, s1 + b.reshape(b.shape[0], -1).sum(axis=-1, keepdims=True)


# t = (relu(in0 + s0)^2 + imm2)^2 — squared so ScalarE can apply 1/sqrt to
# recover aw = 1/(relu(u)^2 + imm2) in a single activation with den-accum.
SQSQ01 = _register_dve_op("SQSQ01_ANT", Spec(
    body=_sq(_sq(_relu(Src0 + C0)) + C2), reference=_ref_sqsq))
# out = relu(in0 + s0) * in1;  accum = s1 + sum(out)   (edge msg * weight)
RELUMR = _register_dve_op("RELUMR_ANT", Spec(
    body=_relu(Src0 + C0) * Src1, accum=_addop, accum_init=C1,
    reference=_ref_relumr))

N = 768
E = 64
MD = 6
NCORES = 8
PI = N // NCORES          # 96 rows per core
NPAIR = PI // 2           # 48 pair-blocks per core
F = N                     # free dim = all j

dt = mybir.dt
AF = mybir.ActivationFunctionType
ALU = mybir.AluOpType

TRACE = False
LAST_EXEC_NS = None
LAST_RESULTS = None

_compiled = {}


def _recip_act(nc, out, in_, bias_const):
    """out = 1/(in_ + bias).  Raw InstActivation: the bass wrapper bans
    Reciprocal generically; measured accuracy here is ~1e-5 relative on the
    positive, well-conditioned inputs this kernel produces."""
    eng = nc.scalar
    ins_ = [eng.lower_ap(in_)]
    for v in (bias_const, 1.0, 0.0):  # bias, scale, alpha
        ins_.append(mybir.ImmediateValue(dtype=dt.float32, value=float(v)))
    return eng.add_instruction(
        mybir.InstActivation(
            name=nc.get_next_instruction_name(),
            func=AF.Reciprocal,
            ins=ins_,
            outs=[eng.lower_ap(out)],
        )
    )


def _build(single=False):
    """single=True builds a 1-core variant with the collective replaced by a
    local DRAM bounce (same DMA shapes) — used for TimelineSim cost modeling."""
    nc = bacc.Bacc("TRN2", target_bir_lowering=False, debug=False,
                   num_devices=1 if single else NCORES)

    def di(name, shape):
        return nc.dram_tensor(name, list(shape), dt.float32,
                              kind="ExternalInput").ap()

    def do(name, shape):
        return nc.dram_tensor(name, list(shape), dt.float32,
                              kind="ExternalOutput").ap()

    def dib(name, shape):
        return nc.dram_tensor(name, list(shape), dt.bfloat16,
                              kind="ExternalInput").ap()

    mT_d = dib("mT", (MD, N))
    ones_d = dib("onesN", (1, N))
    eT0_d = dib("eT0", (E, N))
    mTo_d = di("mT_own", (MD, PI))
    mTob_d = dib("mT_own_bf", (MD, PI))
    eTo_d = di("eT0_own", (E, PI))
    eTob_d = dib("eT0_own_bf", (E, PI))
    W71_d = dib("W71", (71, 768))   # [WBa|WBe|WAa1|WAa2|WCe1|WCe2]
    Wn1_d = di("Wn1", (72, E))      # node-MLP L1, rows reordered [agg|m|m2]
    W65_d = di("W65", (65, 136))    # [Wn2b|Wp2b|Wt2b|Wo2b|Wf1|Wf2b]
    W64_d = di("W64", (64, 197))    # [Wp1|Wt1|Wo1|bn1|bp1|bt1|bo1|bf1]

    posT_d = do("posT", (2, PI))
    tanT_d = do("tanT", (2, PI))
    norT_d = do("norT", (2, PI))
    futT_d = do("futT", (2, PI))
    e2T_d = do("e2T", (E, PI))

    with tile.TileContext(nc) as tc:
        with (
            tc.tile_pool(name="const", bufs=1) as const,
            tc.tile_pool(name="big", bufs=2) as big,
            tc.tile_pool(name="small", bufs=2) as small,
            tc.tile_pool(name="sm2", bufs=3) as sm2,
            tc.tile_pool(name="pe2", bufs=2) as pe2,
            tc.tile_pool(name="pd", bufs=4) as pd,
            tc.tile_pool(name="pawb", bufs=4) as pawb,
            tc.tile_pool(name="ptr", bufs=2) as ptr,
            tc.tile_pool(name="psbt", bufs=2, space="PSUM") as psbt,
            tc.tile_pool(name="psdt", bufs=1, space="PSUM") as psdt,
            tc.tile_pool(name="pssm", bufs=2, space="PSUM") as pssm,
            tc.tile_pool(name="dram", bufs=1, space="DRAM") as dram,
        ):
            # ---- static setup (DMAs spread across engine queues) --------
            rhs_all = const.tile([71, N], dt.bfloat16)  # [m; e; 1] features
            nc.sync.dma_start(rhs_all[0:MD, :], mT_d)
            nc.sync.dma_start(rhs_all[MD:MD + E, :], eT0_d)
            nc.sync.dma_start(rhs_all[70:71, :], ones_d)

            feat_own = const.tile([71, PI], dt.bfloat16)  # own columns
            nc.gpsimd.dma_start(feat_own[0:MD, :], mTob_d)
            nc.gpsimd.dma_start(feat_own[MD:MD + E, :], eTob_d)
            nc.gpsimd.dma_start(feat_own[70:71, :], ones_d[0:1, 0:PI])

            node_inT = const.tile([72, PI], dt.float32)
            nc.gpsimd.dma_start(node_inT[E:E + MD, :], mTo_d)
            nc.gpsimd.dma_start(node_inT[E + MD:72, :], mTo_d[0:2, :])

            eT_own1 = const.tile([E, PI], dt.float32)
            nc.gpsimd.dma_start(eT_own1[:], eTo_d)

            W71 = const.tile([71, 768], dt.bfloat16)
            nc.scalar.dma_start(W71[:], W71_d)
            Wn1 = const.tile([72, E], dt.float32)
            nc.scalar.dma_start(Wn1[:], Wn1_d)
            W65 = const.tile([65, 136], dt.float32)
            nc.scalar.dma_start(W65[:], W65_d)
            W64 = const.tile([64, 197], dt.float32)
            nc.scalar.dma_start(W64[:], W64_d)
            WBa, WBe = W71[:, 0:128], W71[:, 128:256]
            WAa1, WAa2 = W71[:, 256:384], W71[:, 384:512]
            WCe1, WCe2 = W71[:, 512:640], W71[:, 640:768]
            Wn2b, Wf1 = W65[:, 0:64], W65[:, 70:134]
            bn1 = W64[:, 192:193]
            heads = [(W64[:, 0:64], W64[:, 193:194], W65[:, 64:66]),
                     (W64[:, 64:128], W64[:, 194:195], W65[:, 66:68]),
                     (W64[:, 128:192], W64[:, 195:196], W65[:, 68:70])]
            bf1 = W64[:, 196:197]
            Wf2b = W65[:, 134:136]

            e_prev = eT_own1  # [E, PI] tile holding current own embedding
            e2t = None

            for it in range(2):
                # ---- per-iteration projections --------------------------
                btps = psbt.tile([128, 1024], dt.float32)
                nc.tensor.matmul(btps[:, 0:512], WBa, rhs_all[:, 0:512])
                nc.tensor.matmul(btps[:, 512:768], WBa, rhs_all[:, 512:768])
                aps = pssm.tile([128, NPAIR], dt.float32, tag="sm")
                nc.tensor.matmul(aps[:], WAa1, feat_own[:, 0:NPAIR],
                                 start=True, stop=False)
                nc.tensor.matmul(aps[:], WAa2, feat_own[:, NPAIR:PI],
                                 start=False, stop=True)
                BT2 = big.tile([128, F], dt.float32)
                nc.scalar.copy(BT2[:], btps[:, 0:768])
                A_sb = small.tile([128, NPAIR], dt.float32)
                nc.scalar.copy(A_sb[:], aps[:])

                dtps = psdt.tile([128, 1024], dt.float32)
                nc.tensor.matmul(dtps[:, 0:512], WBe, rhs_all[:, 0:512])
                nc.tensor.matmul(dtps[:, 512:768], WBe, rhs_all[:, 512:768])
                cps = pssm.tile([128, NPAIR], dt.float32, tag="sm")
                nc.tensor.matmul(cps[:], WCe1, feat_own[:, 0:NPAIR],
                                 start=True, stop=False)
                nc.tensor.matmul(cps[:], WCe2, feat_own[:, NPAIR:PI],
                                 start=False, stop=True)
                DT2 = big.tile([128, F], dt.float32)
                nc.scalar.copy(DT2[:], dtps[:, 0:768])
                C_sb = small.tile([128, NPAIR], dt.float32)
                nc.scalar.copy(C_sb[:], cps[:])

                num_cols = small.tile([128, NPAIR], dt.float32)
                den_cols = small.tile([128, NPAIR], dt.float32)

                # ---- the N^2 elementwise sweep, 48 pair-blocks ----------
                # software-pipelined: V emits SQSQ for block b+1 before
                # RELUMR for block b, so V never waits on S's rsqrt.
                def emit_sqsq(b):
                    t = pd.tile([128, F], dt.float32, tag="t")
                    nc.vector._custom_dve(SQSQ01, out=t[:], in0=BT2[:],
                                          s0=A_sb[:, b:b + 1], s1=0.0,
                                          imm2=0.01)
                    return t

                def emit_rsqrt(b, t):
                    # aw = 1/sqrt(t) = 1/(relu(u)^2+0.01); den accum is free
                    aw_bf = pawb.tile([128, F], dt.bfloat16, tag="aw")
                    nc.scalar.activation(aw_bf[:], t[:],
                                         AF.Abs_reciprocal_sqrt,
                                         bias=0.0, scale=1.0,
                                         accum_out=den_cols[:, b:b + 1])
                    return aw_bf

                def emit_relumr(b, aw_bf):
                    tr = ptr.tile([128, F], dt.bfloat16, tag="tr")
                    nc.vector._custom_dve(RELUMR, out=tr[:], in0=DT2[:],
                                          in1=aw_bf[:],
                                          s0=C_sb[:, b:b + 1], s1=0.0,
                                          accum_out=num_cols[:, b:b + 1])

                t_prev = emit_sqsq(0)
                aw_prev = emit_rsqrt(0, t_prev)
                for b in range(1, NPAIR):
                    t_cur = emit_sqsq(b)
                    emit_relumr(b - 1, aw_prev)
                    aw_prev = emit_rsqrt(b, t_cur)
                emit_relumr(NPAIR - 1, aw_prev)

                # ---- aggregate + node MLP -------------------------------
                rec_cols = sm2.tile([128, NPAIR], dt.float32)
                nc.vector.reciprocal_approx_fast(rec_cols[:], den_cols[:])
                q_cols = sm2.tile([128, NPAIR], dt.float32)
                nc.vector.tensor_tensor(q_cols[:], num_cols[:], rec_cols[:],
                                        op=ALU.mult)
                qT = sm2.tile([E, PI], dt.float32)
                nc.sync.dma_start(qT[:, 0:NPAIR], q_cols[0:E, :])
                nc.sync.dma_start(qT[:, NPAIR:PI], q_cols[E:128, :])
                # agg -> node_inT rows 0:64 (Wn1 rows reordered to match)
                nc.vector.tensor_tensor(node_inT[0:E, :], qT[:],
                                        e_prev[0:E, :], op=ALU.add)

                nhps = pssm.tile([E, PI], dt.float32, tag="sm")
                nc.tensor.matmul(nhps[:], Wn1[:], node_inT[0:72, :])
                nh = sm2.tile([E + 1, PI], dt.float32)
                nc.scalar.activation(nh[0:E, :], nhps[:], AF.Relu,
                                     bias=bn1, scale=1.0)
                nc.vector.memset(nh[E:E + 1, :], 1.0)
                e2ps = pssm.tile([E, PI], dt.float32, tag="sm")
                nc.tensor.matmul(e2ps[:], Wn2b, nh[0:E + 1, :])
                e2t = pe2.tile([E + 1, PI], dt.float32)
                nc.scalar.copy(e2t[0:E, :], e2ps[:])
                nc.vector.memset(e2t[E:E + 1, :], 1.0)

                if it == 0:
                    e2bf = pe2.tile([E, PI], dt.bfloat16, tag="e2bf")
                    nc.scalar.copy(e2bf[:], e2ps[:])
                    # all-gather updated embeddings for iteration 2
                    ag_in = dram.tile([E, PI], dt.bfloat16)
                    if single:
                        ag_out = dram.tile([E * NCORES, PI], dt.bfloat16)
                    else:
                        ag_out = dram.tile([E * NCORES, PI], dt.bfloat16,
                                           addr_space="Shared")
                    nc.sync.dma_start(ag_in[:], e2bf[:])
                    if single:
                        nc.sync.dma_start(ag_out[0:E, :], ag_in[:])
                    else:
                        nc.gpsimd.collective_compute(
                            "AllGather", ALU.bypass,
                            replica_groups=[list(range(NCORES))],
                            ins=[ag_in.opt()], outs=[ag_out.opt()],
                        )
                    nc.sync.dma_start(
                        rhs_all[MD:MD + E, :].rearrange("k (g c) -> k g c",
                                                        g=NCORES),
                        ag_out[:].rearrange("(g k) c -> k g c", k=E))
                    nc.gpsimd.dma_start(feat_own[MD:MD + E, :], e2bf[:])
                    e_prev = e2t

            # ---- heads ----------------------------------------------------
            for (W1, b1, W2b), out_d in zip(heads, (posT_d, tanT_d, norT_d)):
                hps = pssm.tile([E, PI], dt.float32, tag="sm")
                nc.tensor.matmul(hps[:], W1, e2t[0:E, :])
                h = sm2.tile([E + 1, PI], dt.float32)
                nc.scalar.activation(h[0:E, :], hps[:], AF.Relu,
                                     bias=b1, scale=1.0)
                nc.vector.memset(h[E:E + 1, :], 1.0)
                ops2 = pssm.tile([2, PI], dt.float32, tag="sm")
                nc.tensor.matmul(ops2[:], W2b, h[0:E + 1, :])
                osb = sm2.tile([2, PI], dt.float32)
                nc.scalar.copy(osb[:], ops2[:])
                nc.sync.dma_start(out_d, osb[:])

            # future-position head reads [e2, 1] directly (row E is ones)
            fps = pssm.tile([E, PI], dt.float32, tag="sm")
            nc.tensor.matmul(fps[:], Wf1, e2t[0:E + 1, :])
            fh = sm2.tile([E + 1, PI], dt.float32)
            nc.scalar.activation(fh[0:E, :], fps[:], AF.Relu,
                                 bias=bf1, scale=1.0)
            nc.vector.memset(fh[E:E + 1, :], 1.0)
            fps2 = pssm.tile([2, PI], dt.float32, tag="sm")
            nc.tensor.matmul(fps2[:], Wf2b, fh[0:E + 1, :])
            fsb = sm2.tile([2, PI], dt.float32)
            nc.scalar.copy(fsb[:], fps2[:])
            nc.sync.dma_start(futT_d, fsb[:])

            nc.sync.dma_start(e2T_d, e2t[0:E, :])

    nc.compile()
    return nc


def kernel(neighborhood, neighborhood_embedding, representative_id,
           Wa, ba, We, be, Wn1, bn1, Wn2, bn2,
           Wp1, bp1, Wp2, bp2, Wt1, bt1, Wt2, bt2, Wo1, bo1, Wo2, bo2,
           Wf1, bf1, Wf2, bf2):
    global LAST_EXEC_NS, LAST_RESULTS

    f32 = np.float32
    m = np.asarray(neighborhood, f32)
    e0 = np.asarray(neighborhood_embedding, f32)
    rep = int(np.asarray(representative_id))
    Wa = np.asarray(Wa, f32); ba = np.asarray(ba, f32)
    We = np.asarray(We, f32); be = np.asarray(be, f32)
    Wn1 = np.asarray(Wn1, f32); bn1 = np.asarray(bn1, f32)
    Wn2 = np.asarray(Wn2, f32); bn2 = np.asarray(bn2, f32)

    def blockify(W, b):
        # per-node projection weights for [m; e; 1] features
        WA = np.vstack([W[0:MD] / 10.0, W[MD + MD:MD + MD + E], b[None, :]])
        WB = np.vstack([W[MD:2 * MD], W[2 * MD + E:], np.zeros((1, E), f32)])
        return WA.astype(f32), WB.astype(f32)

    WAblk, WBblk = blockify(Wa, ba)
    WCblk, WDblk = blockify(We, be)
    z = np.zeros_like(WAblk)
    # columns: [WBa(128) | WBe(128) | WAa1(128) | WAa2(128) | WCe1(128) | WCe2(128)]
    W71 = np.concatenate([
        np.concatenate([WBblk, WBblk], axis=1),
        np.concatenate([WDblk, WDblk], axis=1),
        np.concatenate([WAblk, z], axis=1),
        np.concatenate([z, WAblk], axis=1),
        np.concatenate([WCblk, z], axis=1),
        np.concatenate([z, WCblk], axis=1)], axis=1)
    bn1_adj = (bn1 - m[rep, :2] @ Wn1[MD:MD + 2]).reshape(E, 1)
    Wn1_r = np.vstack([Wn1[MD + 2:], Wn1[0:MD], Wn1[MD:MD + 2]])  # [agg|m|m2]
    W65 = np.concatenate([
        np.vstack([Wn2, np.asarray(bn2, f32)[None, :]]),
        np.vstack([np.asarray(Wp2, f32), np.asarray(bp2, f32)[None, :]]),
        np.vstack([np.asarray(Wt2, f32), np.asarray(bt2, f32)[None, :]]),
        np.vstack([np.asarray(Wo2, f32), np.asarray(bo2, f32)[None, :]]),
        np.asarray(Wf1, f32),
        np.vstack([np.asarray(Wf2, f32), np.asarray(bf2, f32)[None, :]])], axis=1)
    W64 = np.concatenate([
        np.asarray(Wp1, f32), np.asarray(Wt1, f32), np.asarray(Wo1, f32),
        bn1_adj,
        np.asarray(bp1, f32).reshape(E, 1),
        np.asarray(bt1, f32).reshape(E, 1),
        np.asarray(bo1, f32).reshape(E, 1),
        np.asarray(bf1, f32).reshape(E, 1)], axis=1)
    import ml_dtypes
    bf16 = ml_dtypes.bfloat16
    shared = {
        "mT": m.T.astype(bf16), "onesN": np.ones((1, N), bf16),
        "eT0": e0.T.astype(bf16), "W71": W71.astype(bf16),
        "Wn1": Wn1_r.astype(f32), "W65": W65.astype(f32),
        "W64": W64.astype(f32),
    }
    shared = {k: np.ascontiguousarray(v) for k, v in shared.items()}

    in_maps = []
    for c in range(NCORES):
        sl = slice(PI * c, PI * (c + 1))
        im = dict(shared)
        im["mT_own"] = np.ascontiguousarray(m.T[:, sl])
        im["mT_own_bf"] = np.ascontiguousarray(m.T[:, sl].astype(bf16))
        im["eT0_own"] = np.ascontiguousarray(e0.T[:, sl])
        im["eT0_own_bf"] = np.ascontiguousarray(e0.T[:, sl].astype(bf16))
        in_maps.append(im)

    if "nc" not in _compiled:
        _compiled["nc"] = _build()
    nc = _compiled["nc"]

    kw = {}
    if TRACE:
        kw = dict(trace=True, trace_cores=list(range(NCORES)),
                  stitch_traces=True)
    res = bass_utils.run_bass_kernel_spmd(
        nc, in_maps, core_ids=list(range(NCORES)), **kw)
    LAST_EXEC_NS = res.exec_time_ns
    LAST_RESULTS = res

    pos = np.concatenate([res.results[c]["posT"].T for c in range(NCORES)])
    tan = np.concatenate([res.results[c]["tanT"].T for c in range(NCORES)])
    nor = np.concatenate([res.results[c]["norT"].T for c in range(NCORES)])
    fut = np.concatenate([res.results[c]["futT"].T for c in range(NCORES)])
    e2 = np.concatenate([res.results[c]["e2T"].T for c in range(NCORES)])

    return (m[rep:rep + 1, :].copy(), e2[rep:rep + 1, :].copy(),
            pos, tan, nor, fut, fut.copy())


def timeline_estimate_ns(trace=False):
    """Cost-model estimate of per-core HW exec time (single-core build,
    collective replaced by a local DRAM bounce)."""
    from concourse.timeline_sim import TimelineSim
    nc = _build(single=True)
    tl = TimelineSim(nc, trace=trace)
    total = tl.simulate()
    return total, tl


# revision 17
# speedup vs baseline: 1.1053x; 1.0367x over previous
"""Trainium2 Bass kernel for nn_LocalProcessing (GNN message passing).

Self-contained: takes the full (unsharded) inputs of reference.setup_inputs(),
shards rows i over 8 NeuronCores, runs a Bass/Tile kernel via
bass_utils.run_bass_kernel_spmd, and returns the full output tuple.

Math: the all-pairs edge feature x[i,j] = [m_i/10, m_j, e_i, e_j] feeds two
Linear+ReLU layers.  x[i,j] @ W decomposes as A_i + B_j with per-node
projections A = [m/10, e, 1] @ [W_mi; W_ei; bias], B = [m, e] @ [W_mj; W_ej],
so the N^2-sized matmul collapses to small per-node matmuls plus elementwise
work over [i, j, k] handled on Scalar/Vector engines:
    aw = 1 / (relu(Aa_i + Ba_j)^2 + 0.01)
    ep = relu(Ae_i + Be_j)
    agg_i = e_i + sum_j(ep*aw) / sum_j(aw)
Each core owns 96 rows i, processed as 48 pair-blocks with SBUF layout
[partitions = (2 rows, 64 feats), free = 768 js].  One AllGather of the
updated embedding runs between the two message-passing iterations.
"""

import numpy as np

import concourse.bass as bass
import concourse.bacc as bacc
import concourse.mybir as mybir
import concourse.tile as tile
import concourse.bass_utils as bass_utils
from operator import add as _addop

from concourse import dve_ops
from concourse.dve_spec import Spec, Src0, Src1, C0, C1, C2, relu as _relu, sq as _sq, lower as _lower
from concourse.dve_spec import _has_src1 as _has_src1
from concourse.dve_uop import DveOpSpec


def _register_dve_op(name, spec):
    """Author a custom DVE op at runtime (tables are generated per-NEFF)."""
    for op in dve_ops.OPS:
        if op.name == name:
            return op
    row = max(dve_ops._SUB_OPCODE_FOR_NAME.values()) + 1
    assert row < 0x20
    dve_ops._SUB_OPCODE_FOR_NAME[name] = row
    shas = {}
    for ver in ("v3", "v4"):
        s = DveOpSpec(name=name, opcode=row, uops=_lower(spec, ver=ver),
                      rd1_en=_has_src1(spec))
        shas[ver] = s.sha(ver)
    op = dve_ops.DveOp(name, spec, subdim=False, uops_sha=shas)
    dve_ops.OPS.append(op)
    dve_ops.CUSTOM_DVE_SPECS[name] = spec
    return op


def _ref_sqsq(in0, in1, s0, s1, imm2):
    r = np.maximum(np.nan_to_num(in0.astype(np.float32) + s0), 0)
    return ((r * r + imm2) ** 2).astype(np.float32)


def _ref_relumr(in0, in1, s0, s1, imm2):
    r = np.maximum(np.nan_to_num(in0.astype(np.float32) + s0), 0)
    b = (r * in1).astype(np.float32)
    return b, s1 + b.reshape(b.shape[0], -1).sum(axis=-1, keepdims=True)


# t = (relu(in0 + s0)^2 + imm2)^2 — squared so ScalarE can apply 1/sqrt to
# recover aw = 1/(relu(u)^2 + imm2) in a single activation with den-accum.
SQSQ01 = _register_dve_op("SQSQ01_ANT", Spec(
    body=_sq(_sq(_relu(Src0 + C0)) + C2), reference=_ref_sqsq))
# out = relu(in0 + s0) * in1;  accum = s1 + sum(out)   (edge msg * weight)
RELUMR = _register_dve_op("RELUMR_ANT", Spec(
    body=_relu(Src0 + C0) * Src1, accum=_addop, accum_init=C1,
    reference=_ref_relumr))

N = 768
E = 64
MD = 6
NCORES = 8
PI = N // NCORES          # 96 rows per core
NPAIR = PI // 2           # 48 pair-blocks per core
F = N                     # free dim = all j

dt = mybir.dt
AF = mybir.ActivationFunctionType
ALU = mybir.AluOpType

TRACE = False
LAST_EXEC_NS = None
LAST_RESULTS = None

_compiled = {}


def _recip_act(nc, out, in_, bias_const):
    """out = 1/(in_ + bias).  Raw InstActivation: the bass wrapper bans
    Reciprocal generically; measured accuracy here is ~1e-5 relative on the
    positive, well-conditioned inputs this kernel produces."""
    eng = nc.scalar
    ins_ = [eng.lower_ap(in_)]
    for v in (bias_const, 1.0, 0.0):  # bias, scale, alpha
        ins_.append(mybir.ImmediateValue(dtype=dt.float32, value=float(v)))
    return eng.add_instruction(
        mybir.InstActivation(
            name=nc.get_next_instruction_name(),
            func=AF.Reciprocal,
            ins=ins_,
            outs=[eng.lower_ap(out)],
        )
    )


def _build(single=False):
    """single=True builds a 1-core variant with the collective replaced by a
    local DRAM bounce (same DMA shapes) — used for TimelineSim cost modeling."""
    nc = bacc.Bacc("TRN2", target_bir_lowering=False, debug=False,
                   num_devices=1 if single else NCORES)

    def di(name, shape):
        return nc.dram_tensor(name, list(shape), dt.float32,
                              kind="ExternalInput").ap()

    def do(name, shape):
        return nc.dram_tensor(name, list(shape), dt.float32,
                              kind="ExternalOutput").ap()

    def dib(name, shape):
        return nc.dram_tensor(name, list(shape), dt.bfloat16,
                              kind="ExternalInput").ap()

    mT_d = dib("mT", (MD, N))
    ones_d = dib("onesN", (1, N))
    eT0_d = dib("eT0", (E, N))
    mTo_d = di("mT_own", (MD, PI))
    mTob_d = dib("mT_own_bf", (MD, PI))
    eTo_d = di("eT0_own", (E, PI))
    eTob_d = dib("eT0_own_bf", (E, PI))
    W71_d = dib("W71", (71, 768))   # [WBa|WBe|WAa1|WAa2|WCe1|WCe2]
    Wn1q_d = di("Wn1q", (128, 128))  # block-diag dup of Wn1 agg rows
    Wn1m_d = di("Wn1m", (8, E))      # Wn1 rows for [m|m2] static features
    W65_d = di("W65", (65, 136))    # [Wn2b|Wp2b|Wt2b|Wo2b|Wf1|Wf2b]
    W64_d = di("W64", (64, 197))    # [Wp1|Wt1|Wo1|bn1|bp1|bt1|bo1|bf1]

    posT_d = do("posT", (2, PI))
    tanT_d = do("tanT", (2, PI))
    norT_d = do("norT", (2, PI))
    futT_d = do("futT", (2, PI))
    e2T_d = do("e2T", (E, PI))

    with tile.TileContext(nc) as tc:
        with (
            tc.tile_pool(name="const", bufs=1) as const,
            tc.tile_pool(name="big", bufs=2) as big,
            tc.tile_pool(name="small", bufs=2) as small,
            tc.tile_pool(name="sm2", bufs=3) as sm2,
            tc.tile_pool(name="pe2", bufs=2) as pe2,
            tc.tile_pool(name="pd", bufs=4) as pd,
            tc.tile_pool(name="pawb", bufs=4) as pawb,
            tc.tile_pool(name="ptr", bufs=2) as ptr,
            tc.tile_pool(name="psbt", bufs=1, space="PSUM") as psbt,
            tc.tile_pool(name="psdt", bufs=1, space="PSUM") as psdt,
            tc.tile_pool(name="pssm", bufs=4, space="PSUM") as pssm,
            tc.tile_pool(name="dram", bufs=1, space="DRAM") as dram,
        ):
            # ---- static setup (DMAs spread across engine queues) --------
            rhs_all = const.tile([71, N], dt.bfloat16)  # [m; e; 1] features
            nc.sync.dma_start(rhs_all[0:MD, :], mT_d)
            nc.sync.dma_start(rhs_all[MD:MD + E, :], eT0_d)
            nc.sync.dma_start(rhs_all[70:71, :], ones_d)

            feat_own = const.tile([71, PI], dt.bfloat16)  # own columns
            nc.gpsimd.dma_start(feat_own[0:MD, :], mTob_d)
            nc.gpsimd.dma_start(feat_own[MD:MD + E, :], eTob_d)
            nc.gpsimd.dma_start(feat_own[70:71, :], ones_d[0:1, 0:PI])

            m_static = const.tile([8, PI], dt.float32)
            nc.gpsimd.dma_start(m_static[0:MD, :], mTo_d)
            nc.gpsimd.dma_start(m_static[MD:8, :], mTo_d[0:2, :])

            eT_own1 = const.tile([E, PI], dt.float32)
            nc.gpsimd.dma_start(eT_own1[:], eTo_d)

            W71 = const.tile([71, 768], dt.bfloat16)
            nc.scalar.dma_start(W71[:], W71_d)
            Wn1q = const.tile([128, 128], dt.float32)
            nc.scalar.dma_start(Wn1q[:], Wn1q_d)
            Wn1m = const.tile([8, E], dt.float32)
            nc.scalar.dma_start(Wn1m[:], Wn1m_d)
            W65 = const.tile([65, 136], dt.float32)
            nc.scalar.dma_start(W65[:], W65_d)
            W64 = const.tile([64, 197], dt.float32)
            nc.scalar.dma_start(W64[:], W64_d)
            WBa, WBe = W71[:, 0:128], W71[:, 128:256]
            WAa1, WAa2 = W71[:, 256:384], W71[:, 384:512]
            WCe1, WCe2 = W71[:, 512:640], W71[:, 640:768]
            Wn2b, Wf1 = W65[:, 0:64], W65[:, 70:134]
            bn1 = W64[:, 192:193]
            heads = [(W64[:, 0:64], W64[:, 193:194], W65[:, 64:66]),
                     (W64[:, 64:128], W64[:, 194:195], W65[:, 66:68]),
                     (W64[:, 128:192], W64[:, 195:196], W65[:, 68:70])]
            bf1 = W64[:, 196:197]
            Wf2b = W65[:, 134:136]

            e_prev = eT_own1  # [E, PI] tile holding current own embedding
            e2t = None

            for it in range(2):
                # ---- per-iteration projections --------------------------
                btps = psbt.tile([128, 1024], dt.float32)
                nc.tensor.matmul(btps[:, 0:512], WBa, rhs_all[:, 0:512])
                nc.tensor.matmul(btps[:, 512:768], WBa, rhs_all[:, 512:768])
                aps = pssm.tile([128, NPAIR], dt.float32, tag="sm")
                nc.tensor.matmul(aps[:], WAa1, feat_own[:, 0:NPAIR],
                                 start=True, stop=False)
                nc.tensor.matmul(aps[:], WAa2, feat_own[:, NPAIR:PI],
                                 start=False, stop=True)
                BT2 = big.tile([128, F], dt.float32)
                nc.scalar.copy(BT2[:], btps[:, 0:768])
                A_sb = small.tile([128, NPAIR], dt.float32)
                nc.scalar.copy(A_sb[:], aps[:])

                dtps = psdt.tile([128, 1024], dt.float32)
                nc.tensor.matmul(dtps[:, 0:512], WBe, rhs_all[:, 0:512])
                nc.tensor.matmul(dtps[:, 512:768], WBe, rhs_all[:, 512:768])
                cps = pssm.tile([128, NPAIR], dt.float32, tag="sm")
                nc.tensor.matmul(cps[:], WCe1, feat_own[:, 0:NPAIR],
                                 start=True, stop=False)
                nc.tensor.matmul(cps[:], WCe2, feat_own[:, NPAIR:PI],
                                 start=False, stop=True)
                DT2 = big.tile([128, F], dt.float32)
                nc.scalar.copy(DT2[:], dtps[:, 0:768])
                C_sb = small.tile([128, NPAIR], dt.float32)
                nc.scalar.copy(C_sb[:], cps[:])

                num_cols = small.tile([128, NPAIR], dt.float32)
                den_cols = small.tile([128, NPAIR], dt.float32)

                # ---- the N^2 elementwise sweep, 48 pair-blocks ----------
                # software-pipelined: V emits SQSQ for block b+1 before
                # RELUMR for block b, so V never waits on S's rsqrt.
                def emit_sqsq(b):
                    t = pd.tile([128, F], dt.float32, tag="t")
                    nc.vector._custom_dve(SQSQ01, out=t[:], in0=BT2[:],
                                          s0=A_sb[:, b:b + 1], s1=0.0,
                                          imm2=0.01)
                    return t

                def emit_rsqrt(b, t):
                    # aw = 1/sqrt(t) = 1/(relu(u)^2+0.01); den accum is free
                    aw_bf = pawb.tile([128, F], dt.bfloat16, tag="aw")
                    nc.scalar.activation(aw_bf[:], t[:],
                                         AF.Abs_reciprocal_sqrt,
                                         bias=0.0, scale=1.0,
                                         accum_out=den_cols[:, b:b + 1])
                    return aw_bf

                def emit_relumr(b, aw_bf):
                    tr = ptr.tile([128, F], dt.bfloat16, tag="tr")
                    nc.vector._custom_dve(RELUMR, out=tr[:], in0=DT2[:],
                                          in1=aw_bf[:],
                                          s0=C_sb[:, b:b + 1], s1=0.0,
                                          accum_out=num_cols[:, b:b + 1])

                t_prev = emit_sqsq(0)
                aw_prev = emit_rsqrt(0, t_prev)
                for b in range(1, NPAIR):
                    t_cur = emit_sqsq(b)
                    emit_relumr(b - 1, aw_prev)
                    aw_prev = emit_rsqrt(b, t_cur)
                emit_relumr(NPAIR - 1, aw_prev)

                # ---- aggregate + node MLP -------------------------------
                # agg = e_prev + num/den feeds only the node MLP, so instead
                # of materializing it, accumulate the three contributions
                # (e_prev, q in pair-column layout, static m features)
                # directly in PSUM.
                rec_cols = sm2.tile([128, NPAIR], dt.float32)
                nc.vector.reciprocal_approx_fast(rec_cols[:], den_cols[:])
                q_cols = sm2.tile([128, NPAIR], dt.float32)
                nc.vector.tensor_tensor(q_cols[:], num_cols[:], rec_cols[:],
                                        op=ALU.mult)
                nhps = pssm.tile([E, PI], dt.float32, tag="sm")
                nc.tensor.matmul(nhps[:], Wn1q[0:E, 0:E], e_prev[0:E, :],
                                 start=True, stop=False, skip_group_check=True)
                nc.tensor.matmul(nhps[:], Wn1m[:], m_static[:],
                                 start=False, stop=False, skip_group_check=True)
                nc.tensor.matmul(nhps[:, 0:NPAIR], Wn1q[:, 0:E], q_cols[:],
                                 start=False, stop=False, skip_group_check=True)
                nc.tensor.matmul(nhps[:, NPAIR:PI], Wn1q[:, E:128], q_cols[:],
                                 start=False, stop=True, skip_group_check=True)
                nh = sm2.tile([E + 1, PI], dt.float32)
                nc.scalar.activation(nh[0:E, :], nhps[:], AF.Relu,
                                     bias=bn1, scale=1.0)
                nc.vector.memset(nh[E:E + 1, :], 1.0)
                e2ps = pssm.tile([E, PI], dt.float32, tag="sm")
                nc.tensor.matmul(e2ps[:], Wn2b, nh[0:E + 1, :])
                e2t = pe2.tile([E + 1, PI], dt.float32)
                nc.scalar.copy(e2t[0:E, :], e2ps[:])
                nc.vector.memset(e2t[E:E + 1, :], 1.0)

                if it == 0:
                    e2bf = pe2.tile([E, PI], dt.bfloat16, tag="e2bf")
                    nc.scalar.copy(e2bf[:], e2ps[:])
                    # all-gather updated embeddings for iteration 2
                    ag_in = dram.tile([E, PI], dt.bfloat16)
                    if single:
                        ag_out = dram.tile([E * NCORES, PI], dt.bfloat16)
                    else:
                        ag_out = dram.tile([E * NCORES, PI], dt.bfloat16,
                                           addr_space="Shared")
                    nc.sync.dma_start(ag_in[:], e2bf[:])
                    if single:
                        nc.sync.dma_start(ag_out[0:E, :], ag_in[:])
                    else:
                        nc.gpsimd.collective_compute(
                            "AllGather", ALU.bypass,
                            replica_groups=[list(range(NCORES))],
                            ins=[ag_in.opt()], outs=[ag_out.opt()],
                        )
                    nc.sync.dma_start(
                        rhs_all[MD:MD + E, :].rearrange("k (g c) -> k g c",
                                                        g=NCORES),
                        ag_out[:].rearrange("(g k) c -> k g c", k=E))
                    nc.gpsimd.dma_start(feat_own[MD:MD + E, :], e2bf[:])
                    e_prev = e2t

            # ---- heads ----------------------------------------------------
            for (W1, b1, W2b), out_d in zip(heads, (posT_d, tanT_d, norT_d)):
                hps = pssm.tile([E, PI], dt.float32, tag="sm")
                nc.tensor.matmul(hps[:], W1, e2t[0:E, :])
                h = sm2.tile([E + 1, PI], dt.float32)
                nc.scalar.activation(h[0:E, :], hps[:], AF.Relu,
                                     bias=b1, scale=1.0)
                nc.vector.memset(h[E:E + 1, :], 1.0)
                ops2 = pssm.tile([2, PI], dt.float32, tag="sm")
                nc.tensor.matmul(ops2[:], W2b, h[0:E + 1, :])
                osb = sm2.tile([2, PI], dt.float32)
                nc.scalar.copy(osb[:], ops2[:])
                nc.sync.dma_start(out_d, osb[:])

            # future-position head reads [e2, 1] directly (row E is ones)
            fps = pssm.tile([E, PI], dt.float32, tag="sm")
            nc.tensor.matmul(fps[:], Wf1, e2t[0:E + 1, :])
            fh = sm2.tile([E + 1, PI], dt.float32)
            nc.scalar.activation(fh[0:E, :], fps[:], AF.Relu,
                                 bias=bf1, scale=1.0)
            nc.vector.memset(fh[E:E + 1, :], 1.0)
            fps2 = pssm.tile([2, PI], dt.float32, tag="sm")
            nc.tensor.matmul(fps2[:], Wf2b, fh[0:E + 1, :])
            fsb = sm2.tile([2, PI], dt.float32)
            nc.scalar.copy(fsb[:], fps2[:])
            nc.sync.dma_start(futT_d, fsb[:])

            nc.sync.dma_start(e2T_d, e2t[0:E, :])

    nc.compile()
    return nc


def kernel(neighborhood, neighborhood_embedding, representative_id,
           Wa, ba, We, be, Wn1, bn1, Wn2, bn2,
           Wp1, bp1, Wp2, bp2, Wt1, bt1, Wt2, bt2, Wo1, bo1, Wo2, bo2,
           Wf1, bf1, Wf2, bf2):
    global LAST_EXEC_NS, LAST_RESULTS

    f32 = np.float32
    m = np.asarray(neighborhood, f32)
    e0 = np.asarray(neighborhood_embedding, f32)
    rep = int(np.asarray(representative_id))
    Wa = np.asarray(Wa, f32); ba = np.asarray(ba, f32)
    We = np.asarray(We, f32); be = np.asarray(be, f32)
    Wn1 = np.asarray(Wn1, f32); bn1 = np.asarray(bn1, f32)
    Wn2 = np.asarray(Wn2, f32); bn2 = np.asarray(bn2, f32)

    def blockify(W, b):
        # per-node projection weights for [m; e; 1] features
        WA = np.vstack([W[0:MD] / 10.0, W[MD + MD:MD + MD + E], b[None, :]])
        WB = np.vstack([W[MD:2 * MD], W[2 * MD + E:], np.zeros((1, E), f32)])
        return WA.astype(f32), WB.astype(f32)

    WAblk, WBblk = blockify(Wa, ba)
    WCblk, WDblk = blockify(We, be)
    z = np.zeros_like(WAblk)
    # columns: [WBa(128) | WBe(128) | WAa1(128) | WAa2(128) | WCe1(128) | WCe2(128)]
    W71 = np.concatenate([
        np.concatenate([WBblk, WBblk], axis=1),
        np.concatenate([WDblk, WDblk], axis=1),
        np.concatenate([WAblk, z], axis=1),
        np.concatenate([z, WAblk], axis=1),
        np.concatenate([WCblk, z], axis=1),
        np.concatenate([z, WCblk], axis=1)], axis=1)
    bn1_adj = (bn1 - m[rep, :2] @ Wn1[MD:MD + 2]).reshape(E, 1)
    Wn1_agg = Wn1[MD + 2:]                      # [64, 64] agg rows
    Wn1q = np.zeros((128, 128), f32)
    Wn1q[0:E, 0:E] = Wn1_agg
    Wn1q[E:128, E:128] = Wn1_agg
    Wn1m = np.vstack([Wn1[0:MD], Wn1[MD:MD + 2]])  # [8, 64]
    W65 = np.concatenate([
        np.vstack([Wn2, np.asarray(bn2, f32)[None, :]]),
        np.vstack([np.asarray(Wp2, f32), np.asarray(bp2, f32)[None, :]]),
        np.vstack([np.asarray(Wt2, f32), np.asarray(bt2, f32)[None, :]]),
        np.vstack([np.asarray(Wo2, f32), np.asarray(bo2, f32)[None, :]]),
        np.asarray(Wf1, f32),
        np.vstack([np.asarray(Wf2, f32), np.asarray(bf2, f32)[None, :]])], axis=1)
    W64 = np.concatenate([
        np.asarray(Wp1, f32), np.asarray(Wt1, f32), np.asarray(Wo1, f32),
        bn1_adj,
        np.asarray(bp1, f32).reshape(E, 1),
        np.asarray(bt1, f32).reshape(E, 1),
        np.asarray(bo1, f32).reshape(E, 1),
        np.asarray(bf1, f32).reshape(E, 1)], axis=1)
    import ml_dtypes
    bf16 = ml_dtypes.bfloat16
    shared = {
        "mT": m.T.astype(bf16), "onesN": np.ones((1, N), bf16),
        "eT0": e0.T.astype(bf16), "W71": W71.astype(bf16),
        "Wn1q": Wn1q, "Wn1m": Wn1m, "W65": W65.astype(f32),
        "W64": W64.astype(f32),
    }
    shared = {k: np.ascontiguousarray(v) for k, v in shared.items()}

    in_maps = []
    for c in range(NCORES):
        sl = slice(PI * c, PI * (c + 1))
        im = dict(shared)
        im["mT_own"] = np.ascontiguousarray(m.T[:, sl])
        im["mT_own_bf"] = np.ascontiguousarray(m.T[:, sl].astype(bf16))
        im["eT0_own"] = np.ascontiguousarray(e0.T[:, sl])
        im["eT0_own_bf"] = np.ascontiguousarray(e0.T[:, sl].astype(bf16))
        in_maps.append(im)

    if "nc" not in _compiled:
        _compiled["nc"] = _build()
    nc = _compiled["nc"]

    kw = {}
    if TRACE:
        kw = dict(trace=True, trace_cores=list(range(NCORES)),
                  stitch_traces=True)
    res = bass_utils.run_bass_kernel_spmd(
        nc, in_maps, core_ids=list(range(NCORES)), **kw)
    LAST_EXEC_NS = res.exec_time_ns
    LAST_RESULTS = res

    pos = np.concatenate([res.results[c]["posT"].T for c in range(NCORES)])
    tan = np.concatenate([res.results[c]["tanT"].T for c in range(NCORES)])
    nor = np.concatenate([res.results[c]["norT"].T for c in range(NCORES)])
    fut = np.concatenate([res.results[c]["futT"].T for c in range(NCORES)])
    e2 = np.concatenate([res.results[c]["e2T"].T for c in range(NCORES)])

    return (m[rep:rep + 1, :].copy(), e2[rep:rep + 1, :].copy(),
            pos, tan, nor, fut, fut.copy())


def timeline_estimate_ns(trace=False):
    """Cost-model estimate of per-core HW exec time (single-core build,
    collective replaced by a local DRAM bounce)."""
    from concourse.timeline_sim import TimelineSim
    nc = _build(single=True)
    tl = TimelineSim(nc, trace=trace)
    total = tl.simulate()
    return total, tl
